# revision 18
# baseline (speedup 1.0000x reference)
"""Trainium2 Bass kernel for the Lineq2v2nano equivariant 2->2 layer.

Math (per sample b):
  out[i,j,f] = relu( x[i,j,:]@W0                                  (op0)
                   + totsum@W1' + bias                            (op1, const over i,j)
                   + rowsum[i]@W2'                                (op2, bcast over j)
                   + rowsum[j]@W3'                                (op3, bcast over i)
                   + delta_ij * (rowsum[i]@W4' + totsum@W5' + diag_bias) )

Kernel strategy (data-parallel, 4 samples per core on 8 cores), v3:
  - everything bf16 on the wire; host pre-permutes x into a PE-ready
    layout and casts; output stored bf16, upcast on host.
  - KEY TRICK: j is processed in blocks of 4, so each main matmul has
    K = 64 x-rows ((j%4) x l) + 17 bias rows (rowsum^T ; ones) = 81,
    N = 128 (4j x 32f). The op2 rowbias and the per-j colbias ride in
    the same contraction, so the output is produced in ONE 4096-col
    stream per sample instead of two (the PE here is clock-gated to
    1.2 GHz, 1 col/cycle, so streamed columns are the wall).
  - the 17 bias rows live in partitions 64:81 of the per-sample x
    tile, written by one stride-0-source SBUF->SBUF DMA replica of
    [rowsum^T ; ones] [17,128] -> [17,4096].
  - colbias (op1+op3+bias, flattened [1,4096] by the cf DMA) is row 80
    of the weight-side operand, double-buffered by sample parity.
  - rowsum via a 5-stage bf16 halving tree over (c,h) (DVE 2x mode)
    plus one K=64 selector matmul -> rowsum^T [l,i] in PSUM.
  - the whole bias chain for sample b+1 runs during sample b's mains
    (its ~6us serial DVE<->PE latency never stalls the PE).
  - relu on ACT/DVE during psum->SBUF eviction; relu'd diagonal rows
    go to a separate small DRAM buffer, merged on host (avoids a
    strided overwrite ordered after the 1MB store).
"""

import os
import sys

sys.path.insert(0, "/opt/trn_rl_repo")

import numpy as np

N_CORES = 8
B, N, L, F = 32, 128, 16, 32
NAVG = 50.0
B_LOC = B // N_CORES  # samples per core

_CACHE = {}

LAST_EXEC_NS = None
LAST_RESULTS = None

# bank index -> eviction engine ("a"=ACT, "v"=DVE); DVE carries the
# rowsum tree so ACT takes most of the eviction work
EVICT = ["a", "a", "v", "a", "a", "a", "v", "a"]


def _build_module():
    import concourse.bass as bass
    import concourse.mybir as mybir
    from concourse import bacc
    from concourse.tile import TileContext, add_dep_helper

    f32 = mybir.dt.float32
    bf16 = mybir.dt.bfloat16
    JF = N * F      # 4096

    nc = bacc.Bacc(None, target_bir_lowering=False)
    # cpack layout: sel 0:16 | w34 16:80 | wtot 80:144 | w0d 144:208
    #               | bcat 208:272
    CP = 272
    x2_h = nc.declare_dram_parameter("x2", [B_LOC, 64, JF], bf16, isOutput=False)
    xdgt_h = nc.declare_dram_parameter("xdgt", [16, B_LOC * 128], bf16, isOutput=False)
    cpack_h = nc.declare_dram_parameter("cpack", [128, CP], bf16, isOutput=False)
    wc_h = nc.declare_dram_parameter("wc", [80, JF], bf16, isOutput=False)
    out_h = nc.declare_dram_parameter("out", [B_LOC, N, JF], bf16, isOutput=True)
    zd_h = nc.declare_dram_parameter("zd", [128, B_LOC * 32], bf16, isOutput=True)

    from contextlib import ExitStack

    with TileContext(nc) as tc, ExitStack() as stack:
        consts = stack.enter_context(tc.tile_pool(name="consts", bufs=1))
        cp0 = consts.tile([128, CP], bf16)
        cl = consts.tile([128, CP], bf16)
        # weight-side operand [block-diag W0 (64) ; W2-tiled (16) ; colflat]
        # triple-buffered by sample parity (row 80 rewritten by the cf DMA;
        # bias chains run two samples ahead of the mains)
        wcp0 = consts.tile([81, JF], bf16)
        wcp1 = consts.tile([81, JF], bf16)
        wcp2 = consts.tile([81, JF], bf16)
        ones = consts.tile([1, 128], bf16)
        xdgt = consts.tile([16, B_LOC * 128], bf16)
        zdall = consts.tile([128, B_LOC * 32], bf16)  # relu'd diagonal rows

        nc.vector.memset(ones[:], 1.0)
        # init loads on the Pool ring; the SP ring is kept free for x loads
        nc.gpsimd.dma_start(out=cp0[:], in_=cpack_h[:])
        nc.gpsimd.dma_start(out=wcp0[0:80, :], in_=wc_h[:])
        nc.gpsimd.dma_start(out=wcp1[0:80, :], in_=wc_h[:])
        nc.gpsimd.dma_start(out=wcp2[0:80, :], in_=wc_h[:])
        nc.gpsimd.dma_start(out=xdgt[:], in_=xdgt_h[:])
        nc.vector.tensor_copy(cl[:], cp0[:])
        o_sel, o_w34, o_wtot, o_w0d, o_bcat = 0, 16, 80, 144, 208
        sel = cl[0:64, o_sel : o_sel + 16]
        w34 = cl[0:16, o_w34 : o_w34 + 64]
        wtot = cl[0:16, o_wtot : o_wtot + 64]
        w0d = cl[0:16, o_w0d : o_w0d + 64]
        bcat = cl[0:1, o_bcat : o_bcat + 64]

        xt_p = stack.enter_context(tc.tile_pool(name="xt", bufs=4))
        osb_p = stack.enter_context(tc.tile_pool(name="osb", bufs=2))
        sm_p = stack.enter_context(tc.tile_pool(name="small", bufs=4))
        ps_o = stack.enter_context(tc.tile_pool(name="ps_o", bufs=6, space="PSUM"))
        ps_s = stack.enter_context(tc.tile_pool(name="ps_s", bufs=2, space="PSUM"))

        # per-sample tiles [128, JF]: rows 0:64 x data (DMA), rows 64:81
        # bias rows (replica DMA); loads staggered below
        xcs = []
        for b in range(B_LOC):
            xc = xt_p.tile([128, JF], bf16, tag="xc")
            xcs.append(xc)
        for b in range(2):
            nc.sync.dma_start(out=xcs[b][0:64, 0:2048], in_=x2_h[b][:, 0:2048])
            nc.sync.dma_start(out=xcs[b][0:64, 2048:4096], in_=x2_h[b][:, 2048:4096])

        def bias_chain(b):
            """Rowsum tree + bias/diag path for sample b; runs one sample
            ahead of the mains so its serial latency never stalls the PE."""
            xc = xcs[b]
            # rowsum over (c,h) (free dim): two half-trees (each starts as
            # soon as its load half lands) + merge, bf16 2x on DVE
            tr = sm_p.tile([64, 2048], bf16, tag="tree")
            for hh in range(2):
                base = hh * 2048
                nc.vector.tensor_add(
                    tr[:, hh * 1024 : hh * 1024 + 1024],
                    xc[0:64, base : base + 1024],
                    xc[0:64, base + 1024 : base + 2048],
                )
            t1 = nc.vector.tensor_add(tr[:, 0:1024], tr[:, 0:1024], tr[:, 1024:2048])
            if b + 2 < B_LOC:
                # stagger: sample b+2's load goes out once b's has landed
                for hh in range(2):
                    ld = nc.sync.dma_start(
                        out=xcs[b + 2][0:64, hh * 2048 : (hh + 1) * 2048],
                        in_=x2_h[b + 2][:, hh * 2048 : (hh + 1) * 2048],
                    )
                    add_dep_helper(ld.ins, t1.ins, sync=True,
                                   reason="stagger load behind consumed sample")
            w = 512
            while w >= 128:
                nc.vector.tensor_add(tr[:, 0:w], tr[:, 0:w], tr[:, w : 2 * w])
                w //= 2
            # S[(jj,l), i] = sum over j==jj mod 4 of x[b,i,j,l]

            # fold the jj partition-sum: rowsum^T[l, i] via selector matmul
            prs = ps_s.tile([16, 128], f32, tag="ps_small")
            nc.tensor.matmul(prs[:], lhsT=sel, rhs=tr[:, 0:128], start=True, stop=True)
            # [rowsum^T ; ones] widened to 1024 by DVE doubling copies, then
            # four plain [17,1024] DMAs fill partitions 64:81 of the x tile
            # (a single stride-0 replicating DMA lands on ONE dma engine at
            # ~20ns per 256B descriptor = ~11us; this shape sprays)
            rstcat = sm_p.tile([17, 1024], bf16, tag="rst")
            nc.vector.memset(rstcat[:, 0:128], 1.0)  # row 16 stays all-ones
            nc.vector.tensor_copy(rstcat[0:16, 0:128], prs[:])
            rst = rstcat[0:16, 0:128]
            for w2 in (128, 256, 512):
                nc.vector.tensor_copy(rstcat[:, w2 : 2 * w2], rstcat[:, 0:w2])
            for q in range(4):
                nc.gpsimd.dma_start(
                    out=xc[64:81, q * 1024 : (q + 1) * 1024], in_=rstcat[:]
                )

            # totsum + tiny matmuls
            totc = sm_p.tile([16, 1], bf16, tag="totc")
            with nc.allow_low_precision(reason="totsum terms are tiny"):
                nc.vector.tensor_reduce(
                    out=totc[:], in_=prs[:], axis=mybir.AxisListType.X,
                    op=mybir.AluOpType.add,
                )
            ptv = ps_s.tile([1, 64], f32, tag="ps_small")
            nc.tensor.matmul(ptv[:], lhsT=totc[:], rhs=wtot, start=True, stop=True)
            tv = sm_p.tile([1, 64], bf16, tag="tv")
            nc.vector.tensor_add(tv[:], ptv[:], bcat)
            tvs = sm_p.tile([1, 32], bf16, tag="tvs")
            nc.vector.tensor_add(tvs[:], tv[0:1, 0:32], tv[0:1, 32:64])

            # cd = [colbias | d]: rowsum@[W3p|W4p] + ones x tv
            pcd = ps_s.tile([128, 64], f32, tag="ps_small")
            nc.tensor.matmul(pcd[:], lhsT=rst, rhs=w34, start=True, stop=False)
            nc.tensor.matmul(pcd[:], lhsT=ones[:], rhs=tv[:], start=False, stop=True)
            cd = sm_p.tile([128, 64], bf16, tag="cd")
            nc.vector.tensor_copy(cd[:], pcd[:])

            # flatten colbias [128, 32] -> row 80 of this sample's wcp,
            # as 4 parallel quarter-DMAs (the full flatten is 128 x 64B
            # descriptors serialized on one dma engine)
            wcp = (wcp0, wcp1, wcp2)[b % 3]
            for q in range(4):
                nc.sync.dma_start(
                    out=wcp[80:81, q * 1024 : (q + 1) * 1024],
                    in_=cd[q * 32 : (q + 1) * 32, 0:32],
                )

            # diagonal rows
            pzd = ps_s.tile([128, 32], f32, tag="ps_small")
            nc.tensor.matmul(pzd[:], lhsT=xdgt[:, b * 128 : (b + 1) * 128],
                             rhs=w0d[:, 0:32], start=True, stop=False)
            nc.tensor.matmul(pzd[:], lhsT=rst, rhs=w0d[:, 32:64], start=False, stop=False)
            nc.tensor.matmul(pzd[:], lhsT=ones[:], rhs=tvs[:], start=False, stop=True)
            nc.scalar.activation(
                out=zdall[:, b * 32 : (b + 1) * 32], in_=pzd[:],
                func=mybir.ActivationFunctionType.Relu,
            )
            return wcp

        chain = {0: bias_chain(0)}

        for b in range(B_LOC):
            xc = xcs[b]
            wcp = chain[b]
            osb = osb_p.tile([128, JF], bf16, tag="osb")

            def bank(s, xc=xc, wcp=wcp, osb=osb):
                po = ps_o.tile([128, 512], f32, tag="po")
                for q in range(4):
                    k = 4 * s + q
                    nc.tensor.matmul(
                        po[:, q * 128 : (q + 1) * 128],
                        lhsT=xc[0:81, k * 128 : (k + 1) * 128],
                        rhs=wcp[0:81, k * 128 : (k + 1) * 128],
                        start=(q == 0),
                        stop=(q == 3),
                    )
                oslab = osb[:, s * 512 : (s + 1) * 512]
                if EVICT[s] == "a":
                    nc.scalar.activation(
                        out=oslab, in_=po[:],
                        func=mybir.ActivationFunctionType.Relu,
                    )
                else:
                    nc.vector.tensor_relu(oslab, po[:])

            for s in range(4):
                bank(s)
            # next sample's bias chain: its small matmuls slot into the PE
            # stream here, its DVE work overlaps this sample's evictions
            if b + 1 < B_LOC:
                chain[b + 1] = bias_chain(b + 1)
            for s in range(4, 8):
                bank(s)

            # store on the otherwise-idle Pool ring; diagonal rows go to
            # their own DRAM buffer (merged on host)
            o0 = out_h[:]
            full_dst = bass.AP(
                tensor=o0.tensor,
                offset=o0.offset + b * N * JF,
                ap=[[JF, 128], [1, JF]],
            )
            nc.gpsimd.dma_start(out=full_dst, in_=osb[:])

        # single tiny store of all relu'd diagonal rows
        nc.sync.dma_start(out=zd_h[:], in_=zdall[:])

    nc.finalize()
    return nc


def _prep_consts(w, bias, diag_bias):
    w = np.asarray(w, np.float32)
    w0 = w[:, 0, :]
    w1s = w[:, 1, :] / NAVG**2
    w2s = w[:, 2, :] / NAVG
    w3s = w[:, 3, :] / NAVG
    w4s = w[:, 4, :] / NAVG
    w5s = w[:, 5, :] / NAVG**2
    import ml_dtypes

    bf16 = ml_dtypes.bfloat16
    CP = 272
    cpack = np.zeros((128, CP), np.float32)
    for jj in range(4):
        cpack[jj * 16 : (jj + 1) * 16, 0:16] = np.eye(16, dtype=np.float32)
    cpack[0:16, 16:80] = np.concatenate([w3s, w4s], 1)
    cpack[0:16, 80:144] = np.concatenate([w1s, w5s], 1)
    cpack[0:16, 144:208] = np.concatenate([w0, w2s + w3s + w4s], 1)
    cpack[0, 208:272] = np.concatenate(
        [np.asarray(bias, np.float32), np.asarray(diag_bias, np.float32)]
    )
    # weight operand: rows 0:64 block-diag W0 over 4 j's, rows 64:80
    # W2-tiled; the pattern repeats every 128 cols (32x)
    blk = np.zeros((64, 128), np.float32)
    for jj in range(4):
        blk[jj * 16 : (jj + 1) * 16, jj * 32 : (jj + 1) * 32] = w0
    wc = np.zeros((80, 128 * 32), np.float32)
    wc[0:64, :] = np.tile(blk, (1, 32))
    wc[64:80, :] = np.tile(w2s, (1, 128))
    return {"cpack": cpack.astype(bf16), "wc": np.ascontiguousarray(wc).astype(bf16)}


def _ensure_profile_hook():
    """Register the NTFF profile hook (the boot path skips it when the
    image lacks antenv.axon_hooks); needed only for trace=True runs."""
    import types

    try:
        from antenv.axon_hooks import get_axon_ntff_profile_hook  # noqa: F401
        return
    except ImportError:
        pass
    import antenv

    mod = types.ModuleType("antenv.axon_hooks")
    mod._hook = None
    mod.set_axon_ntff_profile_hook = lambda h: setattr(mod, "_hook", h)
    mod.get_axon_ntff_profile_hook = lambda: mod._hook
    sys.modules["antenv.axon_hooks"] = mod
    antenv.axon_hooks = mod
    try:
        from trn_agent_boot.trn_boot import _ntff_profile_via_ctypes

        mod._hook = _ntff_profile_via_ctypes("/opt/axon/libaxon_pjrt.so")
    except Exception as e:  # pragma: no cover
        print("profile hook setup failed:", e)


def kernel(inputs, w, bias, diag_bias):
    global LAST_EXEC_NS, LAST_RESULTS
    import ml_dtypes
    from concourse.bass_utils import run_bass_kernel_spmd

    bf16 = ml_dtypes.bfloat16

    if "nc" not in _CACHE:
        _CACHE["nc"] = _build_module()
    nc = _CACHE["nc"]

    x = np.asarray(inputs, np.float32)
    # X2[b, jj*16+l, (2c+h)*128 + i] = x[b, i, 8c+4h+jj, l]
    x2 = np.ascontiguousarray(
        x.reshape(B, N, 16, 2, 4, L).transpose(0, 4, 5, 2, 3, 1)
    ).reshape(B, 64, 4096).astype(bf16)
    # xdgt[b][l, i] = x[b, i, i, l]
    xd = x[:, np.arange(N), np.arange(N), :]                # [B, 128 i, 16 l]
    xdgt = np.ascontiguousarray(xd.transpose(0, 2, 1))      # [B, 16, 128]

    consts = _prep_consts(w, bias, diag_bias)

    in_maps = []
    for c in range(N_CORES):
        m = dict(consts)
        m["x2"] = np.ascontiguousarray(x2[c * B_LOC : (c + 1) * B_LOC])
        m["xdgt"] = np.ascontiguousarray(
            xdgt[c * B_LOC : (c + 1) * B_LOC].transpose(1, 0, 2)
        ).reshape(16, B_LOC * 128).astype(bf16)
        in_maps.append(m)

    trace = bool(int(os.environ.get("KERNEL_TRACE", "0")))
    if trace:
        _ensure_profile_hook()
    res = run_bass_kernel_spmd(nc, in_maps, list(range(N_CORES)), trace=trace)
    LAST_EXEC_NS = res.exec_time_ns
    LAST_RESULTS = res
    out = np.concatenate([res.results[c]["out"] for c in range(N_CORES)], axis=0)
    out = out.reshape(B, N, N, F).astype(np.float32)
    idx = np.arange(N)
    for c in range(N_CORES):
        zd = np.asarray(res.results[c]["zd"], dtype=np.float32)  # [128, B_LOC*32]
        for b in range(B_LOC):
            out[c * B_LOC + b, idx, idx, :] = zd[:, b * 32 : (b + 1) * 32]
    return out
